# revision 26
# baseline (speedup 1.0000x reference)
"""CrossAttention Trainium2 kernel, 8-core SPMD, single-pass fp32r + bf16 attn.

Sharding: core c -> (batch b = c//2, head-group g = c%2).  Each core computes
8 of the 16 heads for one batch: q/k/v projections restricted to its
inner-dim slice [g*512:(g+1)*512], full attention for those heads, and a
partial output projection (contraction over its 512 inner dims).  Host
pre-rounds every input to fp32r and sums the two partial outputs per
batch + bias.

Precision: projections and sim run single-pass fp32r (1 cycle/row on the
PE); exp weights and V run bf16 (same PE rate, half the SBUF).  Measured
rel err ~1.5e-3 vs the 2e-2 gate.  All intermediates (Q^T, K^T, V, attn
out) stay SBUF-resident.

Softmax: the denominator rides the attn@v matmul as a ones-column per head
in the augmented V tiles.  Attn output is copied to SBUF unnormalized; the
8 denominator rows of a jn-block are batched into one [8,512] DVE
reciprocal (free-dim-bound, so batching is ~8x cheaper than per-head) and
the normalization (partition-broadcast + in-place multiply) is issued at
the start of the next jn block, off the PE critical path.

Pipeline: the attention loop over (jn, head) software-pipelines sim+exp
(PE->ACT) against the previous head's attn@v (PE), while qproj for jn+1
and the output projection for jn-1 are interleaved one chunk per head so
the PE stays continuously busy (it only reaches its 2.4 GHz p-state after
3us without a stall).
"""
import sys

sys.path.insert(0, "/opt/trn_rl_repo")

import numpy as np

import concourse.bacc as bacc
import concourse.mybir as mybir
import concourse.tile as tile
from concourse.bass_utils import run_bass_kernel_spmd

# bass_utils imports antenv.axon_hooks when trace=True; the read-only antenv
# package in this image lacks it, so register a no-op stub if missing.
try:
    import antenv.axon_hooks  # noqa: F401
except ImportError:
    import types as _types

    _stub = _types.ModuleType("antenv.axon_hooks")
    _stub.get_axon_ntff_profile_hook = lambda: None
    _stub.set_axon_ntff_profile_hook = lambda h: None
    sys.modules["antenv.axon_hooks"] = _stub

F32 = mybir.dt.float32
F32R = mybir.dt.float32r
BF16 = mybir.dt.bfloat16
EXP = mybir.ActivationFunctionType.Exp

B, N, M = 4, 2048, 1024
QD, CD = 1024, 768
HEADS, DH = 16, 64
INNER = HEADS * DH
HG = 8            # heads per core
IS = HG * DH      # inner slice per core = 512
NC = 8

KQ = QD // 128    # 8  k-tiles in q projection
KC = CD // 128    # 6  k-tiles in k/v projection
NJ = N // 512     # 4  n-chunks
MT = M // 128     # 8  m-tiles
IT = IS // 128    # 4  inner-slice tiles (head pairs)

LAST_RESULTS = None  # stashed BassKernelResults for test.py introspection


def _r(x):
    """Round fp32 -> nearest fp32r (11 explicit mantissa bits)."""
    b = np.ascontiguousarray(x, np.float32).view(np.uint32)
    return (((b.astype(np.uint64) + 0x800) & 0xFFFFF000)
            .astype(np.uint32)).view(np.float32)


def build_nc():
    nc = bacc.Bacc("TRN2", target_bir_lowering=False, debug=False, num_devices=NC)

    def din(name, shape):
        return nc.dram_tensor(name, shape, BF16, kind="ExternalInput").ap()

    xT = din("xT", [QD, N])
    cT = din("cT", [CD, M])
    wq = din("wq", [QD, IS])
    wk = din("wk", [CD, IS])
    wv = din("wv", [CD, IS])
    wo = din("wo", [IS, QD])
    out = nc.dram_tensor("out", [N, QD], F32, kind="ExternalOutput").ap()

    with tile.TileContext(nc) as tc:
        with tc.tile_pool(name="resq", bufs=1) as rq, \
             tc.tile_pool(name="resk", bufs=1) as rk, \
             tc.tile_pool(name="resv", bufs=1) as rv, \
             tc.tile_pool(name="resa", bufs=1) as ra, \
             tc.tile_pool(name="wts", bufs=1) as wt, \
             tc.tile_pool(name="kvw", bufs=1) as kvw, \
             tc.tile_pool(name="xs", bufs=1) as xsp, \
             tc.tile_pool(name="es", bufs=2) as esp, \
             tc.tile_pool(name="sm", bufs=2) as smp, \
             tc.tile_pool(name="obp", bufs=2) as obp, \
             tc.tile_pool(name="ps", bufs=3, space="PSUM") as psp, \
             tc.tile_pool(name="po", bufs=2, space="PSUM") as pop:

            # resident tiles
            qt = [rq.tile([128, N], BF16, tag=f"qt{i}", name=f"qt{i}")
                  for i in range(IT)]          # Q^T, head pair per tile
            kc = [rk.tile([128, M], BF16, tag=f"kc{i}", name=f"kc{i}")
                  for i in range(IT)]          # K^T, head pair per tile
            va = [rv.tile([128, HG * 65], BF16, tag=f"va{i}", name=f"va{i}")
                  for i in range(MT)]          # V + ones col per head
            ao = [ra.tile([128, N], BF16, tag=f"ao{i}", name=f"ao{i}")
                  for i in range(IT)]          # attn output ^T (unnorm, then norm)
            wq_sb = [wt.tile([128, IS], BF16, tag=f"wq{k}", name=f"wq{k}")
                     for k in range(KQ)]
            wo_sb = [wt.tile([128, QD], BF16, tag=f"wo{k}", name=f"wo{k}")
                     for k in range(IT)]
            onesr = wt.tile([128, HG], F32, tag="ones", name="ones")
            nc.vector.memset(onesr[:], 1.0)
            # all-ones operands for cross-partition "copies" (TensorCopy
            # requires matching partition bases; TensorTensor does not)
            ones64 = wt.tile([64, 512], F32, tag="ones64", name="ones64")
            nc.vector.memset(ones64[:], 1.0)

            for k in range(KQ):
                nc.sync.dma_start(wq_sb[k][:], wq[k * 128:(k + 1) * 128, :])

            def load_x(jn):
                nsl = slice(jn * 512, (jn + 1) * 512)
                xh = [xsp.tile([128, 512], BF16, tag=f"x{k}", name=f"x{k}",
                               bufs=2) for k in range(KQ)]
                for k in range(KQ):
                    nc.sync.dma_start(xh[k][:], xT[k * 128:(k + 1) * 128, nsl])
                return xh

            def make_qproj(jn, xh):
                """Two chunks; each fills both halves of one 2-bank psum."""
                nsl = slice(jn * 512, (jn + 1) * 512)

                def chunk(pair):
                    ps2 = psp.tile([128, 1024], F32, tag="mm2", name="mm2")
                    for half in range(2):
                        mi = 2 * pair + half
                        hs = slice(half * 512, (half + 1) * 512)
                        isl = slice(mi * 128, (mi + 1) * 128)
                        for k in range(KQ):
                            nc.tensor.matmul(ps2[:, hs], wq_sb[k][:, isl],
                                             xh[k][:], start=(k == 0),
                                             stop=(k == KQ - 1))
                        nc.vector.tensor_copy(qt[mi][:, nsl], ps2[:, hs])
                return [lambda pair=pair: chunk(pair) for pair in range(2)]

            def make_oproj(jn):
                def chunk(ntl):
                    nt = jn * 4 + ntl
                    tsl = slice(nt * 128, (nt + 1) * 128)
                    ob = obp.tile([128, QD], F32, tag="ob", name="ob", bufs=1)
                    ps2 = psp.tile([128, 1024], F32, tag="mm2", name="mm2")
                    for qh in range(2):
                        qsl = slice(qh * 512, (qh + 1) * 512)
                        for k in range(IT):
                            nc.tensor.matmul(ps2[:, qsl], ao[k][:, tsl],
                                             wo_sb[k][:, qsl],
                                             start=(k == 0), stop=(k == IT - 1))
                    nc.vector.tensor_copy(ob[:], ps2[:])
                    nc.sync.dma_start(out[tsl, :], ob[:])
                return [lambda ntl=ntl: chunk(ntl) for ntl in range(4)]

            # ---------------- attention head (sim + exp) ----------------
            def attn_head(h, jn):
                t, r0 = h // 2, (h % 2) * 64
                nsl = slice(jn * 512, (jn + 1) * 512)
                es = [esp.tile([128, 1024], BF16, tag=f"es{p}", name=f"es{p}",
                               bufs=3) for p in range(4)]
                for p in range(4):
                    ps2 = psp.tile([128, 1024], F32, tag="mm2", name="mm2")
                    for half in range(2):
                        mi = 2 * p + half
                        msl = slice(mi * 128, (mi + 1) * 128)
                        nc.tensor.matmul(ps2[:, half * 512:(half + 1) * 512],
                                         kc[t][r0:r0 + 64, msl],
                                         qt[t][r0:r0 + 64, nsl],
                                         start=True, stop=True)
                    # one exp per psum pair halves the ACT fixed overhead
                    nc.scalar.activation(es[p][:], ps2[:], EXP)
                return es

            def attn_tail(h, jn, es, dsb, row):
                t, r0 = h // 2, (h % 2) * 64
                nsl = slice(jn * 512, (jn + 1) * 512)
                po = pop.tile([65, 512], F32, tag="po", name="po")
                for mi in range(MT):
                    p, half = mi // 2, mi % 2
                    nc.tensor.matmul(po[:], va[mi][:, h * 65:h * 65 + 65],
                                     es[p][:, half * 512:(half + 1) * 512],
                                     start=(mi == 0), stop=(mi == MT - 1))
                nc.vector.tensor_mul(ao[t][r0:r0 + 64, nsl], po[0:64, :],
                                     ones64[:])
                if dsb is None:
                    # final heads: denominator handled early via den_early
                    return
                # denom row -> SBUF staging at partition 0 (TensorTensor
                # allows 32-aligned base remap; TensorCopy/DMA don't apply:
                # copies need matching bases, DMA can't read PSUM), then DMA
                # into dsb partition `row` (DMA remaps partitions freely).
                st = smp.tile([1, 512], F32, tag="st", name="st", bufs=2)
                nc.vector.tensor_mul(st[:], po[64:65, :], ones64[0:1, :])
                nc.sync.dma_start(dsb[row:row + 1, :], st[:])

            def norm_run(jn, dsb, heads, tag):
                """Batch-reciprocal one denom block, normalize in place.
                Issued off the PE critical path; the final block is split
                into quarters so its exposed latency chain is short."""
                nsl = slice(jn * 512, (jn + 1) * 512)
                nh = len(heads)
                rsb = smp.tile([nh, 512], F32, tag=f"rsb{tag}",
                               name=f"rsb{tag}", bufs=1)
                nc.vector.reciprocal(rsb[:], dsb[0:nh, :])
                for i, h in enumerate(heads):
                    t, r0 = h // 2, (h % 2) * 64
                    rfh = smp.tile([1, 512], F32, tag="rfh", name="rfh",
                                   bufs=2)
                    nc.sync.dma_start(rfh[:], rsb[i:i + 1, :])
                    pbs = smp.tile([128, 512], F32, tag="pbs", name="pbs",
                                   bufs=4)
                    nc.gpsimd.partition_broadcast(pbs[:], rfh[:])
                    nc.vector.tensor_mul(ao[t][r0:r0 + 64, nsl],
                                         ao[t][r0:r0 + 64, nsl],
                                         pbs[r0:r0 + 64, :])

            def den_early(h, jn, es, rf):
                """Early softmax denominator for a final head: ones-column
                matmul group + reciprocal + broadcast, all overlapped with
                the head's attn@v, so only the normalize multiply remains
                on the tail critical path."""
                ps2d = psp.tile([128, 1024], F32, tag="mm2", name="mm2")
                for mi in range(MT):
                    p, half = mi // 2, mi % 2
                    nc.tensor.matmul(ps2d[0:1, 0:512],
                                     va[mi][:, h * 65 + 64:h * 65 + 65],
                                     es[p][:, half * 512:(half + 1) * 512],
                                     start=(mi == 0), stop=(mi == MT - 1))
                nc.vector.reciprocal(rf[:], ps2d[0:1, 0:512])
                pbs = smp.tile([128, 512], F32, tag="pbs", name="pbs",
                               bufs=4)
                nc.gpsimd.partition_broadcast(pbs[:], rf[:])
                return pbs

            # ---------------- prelude ----------------
            # qproj(0), then kproj (K^T complete), then head 0's sim+exp so
            # the Scalar engine starts early, then vproj.
            with nc.named_scope("qproj"):
                xh0 = load_x(0)
                for c in make_qproj(0, xh0):
                    c()

            wk_sb = [kvw.tile([128, IS], BF16, tag=f"wk{k}", name=f"wk{k}")
                     for k in range(KC)]
            wv_sb = [kvw.tile([128, IS], BF16, tag=f"wv{k}", name=f"wv{k}")
                     for k in range(KC)]
            for k in range(KC):
                ksl = slice(k * 128, (k + 1) * 128)
                nc.sync.dma_start(wk_sb[k][:], wk[ksl, :])
                nc.sync.dma_start(wv_sb[k][:], wv[ksl, :])
            chs = []
            for jm in range(M // 512):
                msl = slice(jm * 512, (jm + 1) * 512)
                ch = [xsp.tile([128, 512], BF16, tag=f"c{k}", name=f"c{k}",
                               bufs=2) for k in range(KC)]
                for k in range(KC):
                    ksl = slice(k * 128, (k + 1) * 128)
                    nc.sync.dma_start(ch[k][:], cT[ksl, msl])
                chs.append(ch)

            with nc.named_scope("kproj"):
                for jm in range(M // 512):
                    msl = slice(jm * 512, (jm + 1) * 512)
                    ch = chs[jm]
                    for pair in range(2):
                        ps2 = psp.tile([128, 1024], F32, tag="mm2", name="mm2")
                        for half in range(2):
                            mi = 2 * pair + half
                            hs = slice(half * 512, (half + 1) * 512)
                            isl = slice(mi * 128, (mi + 1) * 128)
                            for k in range(KC):
                                nc.tensor.matmul(ps2[:, hs], wk_sb[k][:, isl],
                                                 ch[k][:], start=(k == 0),
                                                 stop=(k == KC - 1))
                            nc.vector.tensor_copy(kc[mi][:, msl], ps2[:, hs])

            dsbA0 = smp.tile([4, 512], F32, tag="dsbA", name="dsbA", bufs=1)
            dsbB0 = smp.tile([4, 512], F32, tag="dsbB", name="dsbB", bufs=1)
            with nc.named_scope("attn"):
                es00 = attn_head(0, 0)

            with nc.named_scope("vproj"):
                for jm in range(M // 512):
                    ch = chs[jm]
                    for pair in range(2):
                        ps2 = psp.tile([128, 1024], F32, tag="mm2", name="mm2")
                        for half in range(2):
                            mt = 2 * pair + half
                            mi2 = jm * 4 + mt
                            hs = slice(half * 512, (half + 1) * 512)
                            tsl = slice(mt * 128, (mt + 1) * 128)
                            for k in range(KC):
                                nc.tensor.matmul(ps2[:, hs], ch[k][:, tsl],
                                                 wv_sb[k][:],
                                                 start=(k == 0),
                                                 stop=(k == KC - 1))
                            hcol = va[mi2][:].rearrange("p (h c) -> p h c",
                                                        c=65)
                            psv = ps2[:, hs].rearrange("p (h c) -> p h c",
                                                       c=64)
                            nc.vector.tensor_copy(hcol[:, :, 0:64], psv[:])
                            nc.vector.tensor_copy(hcol[:, :, 64], onesr[:])

            for k in range(IT):
                nc.sync.dma_start(wo_sb[k][:], wo[k * 128:(k + 1) * 128, :])

            # -------- attention + interleaved q/o proj main loop --------
            with nc.named_scope("attn"):
                pend = (0, 0, es00, dsbA0, 0)
                xh_next = None
                dsbB_prev = None
                hold = None
                for jn in range(NJ):
                    if jn == 0:
                        dsbA, dsbB = dsbA0, dsbB0
                    else:
                        dsbA = smp.tile([4, 512], F32, tag="dsbA",
                                        name="dsbA", bufs=1)
                        dsbB = smp.tile([4, 512], F32, tag="dsbB",
                                        name="dsbB", bufs=1)
                    last = jn == NJ - 1
                    if last:
                        dsbC = smp.tile([2, 512], F32, tag="dsbC",
                                        name="dsbC", bufs=1)
                        rfd = {h: smp.tile([1, 512], F32, tag=f"rft{h}",
                                           name=f"rft{h}", bufs=1)
                               for h in (6, 7)}
                        pbd = {}
                    exq = []
                    if jn + 1 < NJ:
                        if xh_next is None:
                            xh_next = load_x(jn + 1)
                        exq = make_qproj(jn + 1, xh_next)
                    exo = make_oproj(jn - 1) if jn > 0 else []
                    if last:
                        hold = exo.pop()   # PE filler for the final norm chain
                    for h in range(HG):
                        if jn == 0 and h == 0:
                            continue       # pre-issued in the prelude
                        es = attn_head(h, jn)
                        if pend is not None:
                            attn_tail(*pend)
                        if h < 4:
                            tgt = (dsbA, h)
                        elif not last:
                            tgt = (dsbB, h - 4)
                        elif h < 6:
                            tgt = (dsbC, h - 4)
                        else:
                            pbd[h] = den_early(h, jn, es, rfd[h])
                            tgt = (None, None)
                        pend = (h, jn, es, tgt[0], tgt[1])
                        if h == 0 and dsbB_prev is not None:
                            # tail(7, jn-1) flushed above -> heads 4-7 done
                            norm_run(jn - 1, dsbB_prev, [4, 5, 6, 7], "B")
                        if h == 5:
                            # tail(3, jn) flushed at h=4 -> heads 0-3 done
                            norm_run(jn, dsbA, [0, 1, 2, 3], "A")
                        if last and h == 7:
                            # tails 4,5 flushed at h=5,6
                            norm_run(jn, dsbC, [4, 5], "C")
                        if h >= 2 and h < 4:
                            if exq:
                                exq.pop(0)()
                                if not exq:
                                    # last qproj(jn+1) chunk issued ->
                                    # x(jn+2) DMA can queue behind its reads
                                    xh_next = (load_x(jn + 2)
                                               if jn + 2 < NJ else None)
                        elif exo:
                            exo.pop(0)()
                    dsbB_prev = dsbB
                attn_tail(*pend)
                nsl_l = slice((NJ - 1) * 512, NJ * 512)
                for h in (6, 7):
                    t, r0 = h // 2, (h % 2) * 64
                    nc.vector.tensor_mul(ao[t][r0:r0 + 64, nsl_l],
                                         ao[t][r0:r0 + 64, nsl_l],
                                         pbd[h][r0:r0 + 64, :])
                hold()

            with nc.named_scope("oproj"):
                for c in make_oproj(NJ - 1):
                    c()

    nc.compile()
    return nc


_NC_CACHE = None


def kernel(x, context, Wq, Wk, Wv, Wo, bo, _trace=False):
    global _NC_CACHE, LAST_RESULTS
    x = np.asarray(x, np.float32)
    context = np.asarray(context, np.float32)
    scale = np.float32(DH ** -0.5)

    if _NC_CACHE is None:
        _NC_CACHE = build_nc()
    nc = _NC_CACHE

    import ml_dtypes
    bf16 = ml_dtypes.bfloat16

    def _b(a):
        return np.ascontiguousarray(np.asarray(a, np.float32)).astype(bf16)

    in_maps = []
    for c in range(NC):
        b, g = c // 2, c % 2
        sl = slice(g * IS, (g + 1) * IS)
        m = {
            "xT": _b(x[b].T),
            "cT": _b(context[b].T),
            "wq": _b(np.asarray(Wq, np.float32)[:, sl] * scale),
            "wk": _b(np.asarray(Wk, np.float32)[:, sl]),
            "wv": _b(np.asarray(Wv, np.float32)[:, sl]),
            "wo": _b(np.asarray(Wo, np.float32)[sl, :]),
        }
        in_maps.append(m)
    res = run_bass_kernel_spmd(nc, in_maps, core_ids=list(range(NC)),
                               trace=_trace)
    LAST_RESULTS = res
    out = np.empty((B, N, QD), np.float32)
    bo32 = np.asarray(bo, np.float32)
    for b in range(B):
        out[b] = res.results[2 * b]["out"] + res.results[2 * b + 1]["out"] + bo32
    return out


# revision 27
# speedup vs baseline: 1.0663x; 1.0663x over previous
"""CrossAttention Trainium2 kernel, 8-core SPMD, single-pass fp32r + bf16 attn.

Sharding: core c -> (batch b = c//2, head-group g = c%2).  Each core computes
8 of the 16 heads for one batch: q/k/v projections restricted to its
inner-dim slice [g*512:(g+1)*512], full attention for those heads, and a
partial output projection (contraction over its 512 inner dims).  Host
pre-rounds every input to fp32r and sums the two partial outputs per
batch + bias.

Precision: projections and sim run single-pass fp32r (1 cycle/row on the
PE); exp weights and V run bf16 (same PE rate, half the SBUF).  Measured
rel err ~1.5e-3 vs the 2e-2 gate.  All intermediates (Q^T, K^T, V, attn
out) stay SBUF-resident.

Softmax: the denominator rides the attn@v matmul as a ones-column per head
in the augmented V tiles.  Attn output is copied to SBUF unnormalized; the
8 denominator rows of a jn-block are batched into one [8,512] DVE
reciprocal (free-dim-bound, so batching is ~8x cheaper than per-head) and
the normalization (partition-broadcast + in-place multiply) is issued at
the start of the next jn block, off the PE critical path.

Pipeline: the attention loop over (jn, head) software-pipelines sim+exp
(PE->ACT) against the previous head's attn@v (PE), while qproj for jn+1
and the output projection for jn-1 are interleaved one chunk per head so
the PE stays continuously busy (it only reaches its 2.4 GHz p-state after
3us without a stall).
"""
import sys

sys.path.insert(0, "/opt/trn_rl_repo")

import numpy as np

import concourse.bacc as bacc
import concourse.mybir as mybir
import concourse.tile as tile
from concourse.bass_utils import run_bass_kernel_spmd

# bass_utils imports antenv.axon_hooks when trace=True; the read-only antenv
# package in this image lacks it, so register a no-op stub if missing.
try:
    import antenv.axon_hooks  # noqa: F401
except ImportError:
    import types as _types

    _stub = _types.ModuleType("antenv.axon_hooks")
    _stub.get_axon_ntff_profile_hook = lambda: None
    _stub.set_axon_ntff_profile_hook = lambda h: None
    sys.modules["antenv.axon_hooks"] = _stub

F32 = mybir.dt.float32
F32R = mybir.dt.float32r
BF16 = mybir.dt.bfloat16
EXP = mybir.ActivationFunctionType.Exp

B, N, M = 4, 2048, 1024
QD, CD = 1024, 768
HEADS, DH = 16, 64
INNER = HEADS * DH
HG = 8            # heads per core
IS = HG * DH      # inner slice per core = 512
NC = 8

KQ = QD // 128    # 8  k-tiles in q projection
KC = CD // 128    # 6  k-tiles in k/v projection
NJ = N // 512     # 4  n-chunks
MT = M // 128     # 8  m-tiles
IT = IS // 128    # 4  inner-slice tiles (head pairs)

LAST_RESULTS = None  # stashed BassKernelResults for test.py introspection


def _r(x):
    """Round fp32 -> nearest fp32r (11 explicit mantissa bits)."""
    b = np.ascontiguousarray(x, np.float32).view(np.uint32)
    return (((b.astype(np.uint64) + 0x800) & 0xFFFFF000)
            .astype(np.uint32)).view(np.float32)


def build_nc():
    nc = bacc.Bacc("TRN2", target_bir_lowering=False, debug=False, num_devices=NC)

    def din(name, shape):
        return nc.dram_tensor(name, shape, BF16, kind="ExternalInput").ap()

    xT = din("xT", [QD, N])
    cT = din("cT", [CD, M])
    wq = din("wq", [QD, IS])
    wk = din("wk", [CD, IS])
    wv = din("wv", [CD, IS])
    wo = din("wo", [IS, QD])
    out = nc.dram_tensor("out", [N, QD], F32, kind="ExternalOutput").ap()

    with tile.TileContext(nc) as tc:
        with tc.tile_pool(name="resq", bufs=1) as rq, \
             tc.tile_pool(name="resk", bufs=1) as rk, \
             tc.tile_pool(name="resv", bufs=1) as rv, \
             tc.tile_pool(name="resa", bufs=1) as ra, \
             tc.tile_pool(name="wts", bufs=1) as wt, \
             tc.tile_pool(name="kvw", bufs=1) as kvw, \
             tc.tile_pool(name="xs", bufs=1) as xsp, \
             tc.tile_pool(name="es", bufs=2) as esp, \
             tc.tile_pool(name="sm", bufs=2) as smp, \
             tc.tile_pool(name="obp", bufs=2) as obp, \
             tc.tile_pool(name="ps", bufs=3, space="PSUM") as psp, \
             tc.tile_pool(name="po", bufs=2, space="PSUM") as pop:

            # resident tiles
            qt = [rq.tile([128, N], BF16, tag=f"qt{i}", name=f"qt{i}")
                  for i in range(IT)]          # Q^T, head pair per tile
            kc = [rk.tile([128, M], BF16, tag=f"kc{i}", name=f"kc{i}")
                  for i in range(IT)]          # K^T, head pair per tile
            va = [rv.tile([128, HG * 65], BF16, tag=f"va{i}", name=f"va{i}")
                  for i in range(MT)]          # V + ones col per head
            ao = [ra.tile([128, N], BF16, tag=f"ao{i}", name=f"ao{i}")
                  for i in range(IT)]          # attn output ^T (unnorm, then norm)
            wq_sb = [wt.tile([128, IS], BF16, tag=f"wq{k}", name=f"wq{k}")
                     for k in range(KQ)]
            wo_sb = [wt.tile([128, QD], BF16, tag=f"wo{k}", name=f"wo{k}")
                     for k in range(IT)]
            onesr = wt.tile([128, HG], F32, tag="ones", name="ones")
            nc.vector.memset(onesr[:], 1.0)
            # all-ones operands for cross-partition "copies" (TensorCopy
            # requires matching partition bases; TensorTensor does not)
            ones64 = wt.tile([64, 512], F32, tag="ones64", name="ones64")
            nc.vector.memset(ones64[:], 1.0)

            for k in range(KQ):
                nc.sync.dma_start(wq_sb[k][:], wq[k * 128:(k + 1) * 128, :])

            def load_x(jn):
                nsl = slice(jn * 512, (jn + 1) * 512)
                xh = [xsp.tile([128, 512], BF16, tag=f"x{k}", name=f"x{k}",
                               bufs=2) for k in range(KQ)]
                for k in range(KQ):
                    nc.sync.dma_start(xh[k][:], xT[k * 128:(k + 1) * 128, nsl])
                return xh

            def make_qproj(jn, xh):
                """Two chunks; each fills both halves of one 2-bank psum."""
                nsl = slice(jn * 512, (jn + 1) * 512)

                def chunk(pair):
                    ps2 = psp.tile([128, 1024], F32, tag="mm2", name="mm2")
                    for half in range(2):
                        mi = 2 * pair + half
                        hs = slice(half * 512, (half + 1) * 512)
                        isl = slice(mi * 128, (mi + 1) * 128)
                        for k in range(KQ):
                            nc.tensor.matmul(ps2[:, hs], wq_sb[k][:, isl],
                                             xh[k][:], start=(k == 0),
                                             stop=(k == KQ - 1))
                        nc.vector.tensor_copy(qt[mi][:, nsl], ps2[:, hs])
                return [lambda pair=pair: chunk(pair) for pair in range(2)]

            def make_oproj(jn):
                def chunk(ntl):
                    nt = jn * 4 + ntl
                    tsl = slice(nt * 128, (nt + 1) * 128)
                    ob = obp.tile([128, QD], F32, tag="ob", name="ob", bufs=1)
                    ps2 = psp.tile([128, 1024], F32, tag="mm2", name="mm2")
                    for qh in range(2):
                        qsl = slice(qh * 512, (qh + 1) * 512)
                        for k in range(IT):
                            nc.tensor.matmul(ps2[:, qsl], ao[k][:, tsl],
                                             wo_sb[k][:, qsl],
                                             start=(k == 0), stop=(k == IT - 1))
                    nc.vector.tensor_copy(ob[:], ps2[:])
                    nc.sync.dma_start(out[tsl, :], ob[:])
                return [lambda ntl=ntl: chunk(ntl) for ntl in range(4)]

            # ---------------- attention head (sim + exp) ----------------
            def attn_head(h, jn):
                t, r0 = h // 2, (h % 2) * 64
                nsl = slice(jn * 512, (jn + 1) * 512)
                es = [esp.tile([128, 1024], BF16, tag=f"es{p}", name=f"es{p}",
                               bufs=3) for p in range(4)]
                for p in range(4):
                    ps2 = psp.tile([128, 1024], F32, tag="mm2", name="mm2")
                    for half in range(2):
                        mi = 2 * p + half
                        msl = slice(mi * 128, (mi + 1) * 128)
                        nc.tensor.matmul(ps2[:, half * 512:(half + 1) * 512],
                                         kc[t][r0:r0 + 64, msl],
                                         qt[t][r0:r0 + 64, nsl],
                                         start=True, stop=True)
                    # one exp per psum pair halves the ACT fixed overhead
                    nc.scalar.activation(es[p][:], ps2[:], EXP)
                return es

            def attn_tail(h, jn, es, dsb, row):
                t, r0 = h // 2, (h % 2) * 64
                nsl = slice(jn * 512, (jn + 1) * 512)
                po = pop.tile([65, 512], F32, tag="po", name="po")
                for mi in range(MT):
                    p, half = mi // 2, mi % 2
                    nc.tensor.matmul(po[:], va[mi][:, h * 65:h * 65 + 65],
                                     es[p][:, half * 512:(half + 1) * 512],
                                     start=(mi == 0), stop=(mi == MT - 1))
                nc.vector.tensor_mul(ao[t][r0:r0 + 64, nsl], po[0:64, :],
                                     ones64[:])
                if dsb is None:
                    # final heads: denominator handled early via den_early
                    return
                # denom row -> SBUF staging at partition 0 (TensorTensor
                # allows 32-aligned base remap; TensorCopy/DMA don't apply:
                # copies need matching bases, DMA can't read PSUM), then DMA
                # into dsb partition `row` (DMA remaps partitions freely).
                st = smp.tile([1, 512], F32, tag="st", name="st", bufs=2)
                nc.vector.tensor_mul(st[:], po[64:65, :], ones64[0:1, :])
                nc.sync.dma_start(dsb[row:row + 1, :], st[:])

            def norm_run(jn, dsb, heads, tag):
                """Batch-reciprocal one denom block, normalize in place.
                Issued off the PE critical path; the final block is split
                into quarters so its exposed latency chain is short."""
                nsl = slice(jn * 512, (jn + 1) * 512)
                nh = len(heads)
                rsb = smp.tile([nh, 512], F32, tag=f"rsb{tag}",
                               name=f"rsb{tag}", bufs=1)
                nc.vector.reciprocal(rsb[:], dsb[0:nh, :])
                for i, h in enumerate(heads):
                    t, r0 = h // 2, (h % 2) * 64
                    rfh = smp.tile([1, 512], F32, tag="rfh", name="rfh",
                                   bufs=2)
                    nc.sync.dma_start(rfh[:], rsb[i:i + 1, :])
                    pbs = smp.tile([128, 512], F32, tag="pbs", name="pbs",
                                   bufs=4)
                    nc.gpsimd.partition_broadcast(pbs[:], rfh[:])
                    nc.vector.tensor_mul(ao[t][r0:r0 + 64, nsl],
                                         ao[t][r0:r0 + 64, nsl],
                                         pbs[r0:r0 + 64, :])

            def den_early(h, jn, es, rf):
                """Early softmax denominator for a final head: ones-column
                matmul group + reciprocal + broadcast, all overlapped with
                the head's attn@v, so only the normalize multiply remains
                on the tail critical path."""
                ps2d = psp.tile([128, 1024], F32, tag="mm2", name="mm2")
                for mi in range(MT):
                    p, half = mi // 2, mi % 2
                    nc.tensor.matmul(ps2d[0:1, 0:512],
                                     va[mi][:, h * 65 + 64:h * 65 + 65],
                                     es[p][:, half * 512:(half + 1) * 512],
                                     start=(mi == 0), stop=(mi == MT - 1))
                nc.vector.reciprocal(rf[:], ps2d[0:1, 0:512])
                pbs = smp.tile([128, 512], F32, tag="pbs", name="pbs",
                               bufs=4)
                nc.gpsimd.partition_broadcast(pbs[:], rf[:])
                return pbs

            # ---------------- prelude ----------------
            # qproj(0), then kproj (K^T complete), then head 0's sim+exp so
            # the Scalar engine starts early, then vproj.
            with nc.named_scope("qproj"):
                xh0 = load_x(0)
                for c in make_qproj(0, xh0):
                    c()

            wk_sb = [kvw.tile([128, IS], BF16, tag=f"wk{k}", name=f"wk{k}")
                     for k in range(KC)]
            wv_sb = [kvw.tile([128, IS], BF16, tag=f"wv{k}", name=f"wv{k}")
                     for k in range(KC)]
            for k in range(KC):
                ksl = slice(k * 128, (k + 1) * 128)
                nc.sync.dma_start(wk_sb[k][:], wk[ksl, :])
                nc.sync.dma_start(wv_sb[k][:], wv[ksl, :])
            chs = []
            for jm in range(M // 512):
                msl = slice(jm * 512, (jm + 1) * 512)
                ch = [xsp.tile([128, 512], BF16, tag=f"c{k}", name=f"c{k}",
                               bufs=2) for k in range(KC)]
                for k in range(KC):
                    ksl = slice(k * 128, (k + 1) * 128)
                    nc.sync.dma_start(ch[k][:], cT[ksl, msl])
                chs.append(ch)

            with nc.named_scope("kproj"):
                for jm in range(M // 512):
                    msl = slice(jm * 512, (jm + 1) * 512)
                    ch = chs[jm]
                    for pair in range(2):
                        ps2 = psp.tile([128, 1024], F32, tag="mm2", name="mm2")
                        for half in range(2):
                            mi = 2 * pair + half
                            hs = slice(half * 512, (half + 1) * 512)
                            isl = slice(mi * 128, (mi + 1) * 128)
                            for k in range(KC):
                                nc.tensor.matmul(ps2[:, hs], wk_sb[k][:, isl],
                                                 ch[k][:], start=(k == 0),
                                                 stop=(k == KC - 1))
                            nc.vector.tensor_copy(kc[mi][:, msl], ps2[:, hs])

            dsbA0 = smp.tile([4, 512], F32, tag="dsbA", name="dsbA", bufs=1)
            dsbB0 = smp.tile([4, 512], F32, tag="dsbB", name="dsbB", bufs=1)
            with nc.named_scope("attn"):
                es00 = attn_head(0, 0)

            with nc.named_scope("vproj"):
                for jm in range(M // 512):
                    ch = chs[jm]
                    for pair in range(2):
                        ps2 = psp.tile([128, 1024], F32, tag="mm2", name="mm2")
                        for half in range(2):
                            mt = 2 * pair + half
                            mi2 = jm * 4 + mt
                            hs = slice(half * 512, (half + 1) * 512)
                            tsl = slice(mt * 128, (mt + 1) * 128)
                            for k in range(KC):
                                nc.tensor.matmul(ps2[:, hs], ch[k][:, tsl],
                                                 wv_sb[k][:],
                                                 start=(k == 0),
                                                 stop=(k == KC - 1))
                            hcol = va[mi2][:].rearrange("p (h c) -> p h c",
                                                        c=65)
                            psv = ps2[:, hs].rearrange("p (h c) -> p h c",
                                                       c=64)
                            nc.vector.tensor_copy(hcol[:, :, 0:64], psv[:])
                            nc.vector.tensor_copy(hcol[:, :, 64], onesr[:])

            for k in range(IT):
                nc.sync.dma_start(wo_sb[k][:], wo[k * 128:(k + 1) * 128, :])

            # -------- attention + interleaved q/o proj main loop --------
            with nc.named_scope("attn"):
                pend = (0, 0, es00, dsbA0, 0)
                xh_next = None
                dsbB_prev = None
                hold = None
                for jn in range(NJ):
                    if jn == 0:
                        dsbA, dsbB = dsbA0, dsbB0
                    else:
                        dsbA = smp.tile([4, 512], F32, tag="dsbA",
                                        name="dsbA", bufs=1)
                        dsbB = smp.tile([4, 512], F32, tag="dsbB",
                                        name="dsbB", bufs=1)
                    last = jn == NJ - 1
                    if last:
                        dsbC = smp.tile([2, 512], F32, tag="dsbC",
                                        name="dsbC", bufs=1)
                        rfd = {h: smp.tile([1, 512], F32, tag=f"rft{h}",
                                           name=f"rft{h}", bufs=1)
                               for h in (6, 7)}
                        pbd = {}
                    exq = []
                    if jn + 1 < NJ:
                        if xh_next is None:
                            xh_next = load_x(jn + 1)
                        exq = make_qproj(jn + 1, xh_next)
                    exo = make_oproj(jn - 1) if jn > 0 else []
                    if last:
                        hold = exo.pop()   # PE filler for the final norm chain
                    for h in range(HG):
                        if jn == 0 and h == 0:
                            continue       # pre-issued in the prelude
                        es = attn_head(h, jn)
                        if pend is not None:
                            attn_tail(*pend)
                        if h < 4:
                            tgt = (dsbA, h)
                        elif not last:
                            tgt = (dsbB, h - 4)
                        elif h < 6:
                            tgt = (dsbC, h - 4)
                        else:
                            pbd[h] = den_early(h, jn, es, rfd[h])
                            tgt = (None, None)
                        pend = (h, jn, es, tgt[0], tgt[1])
                        if h == 0 and dsbB_prev is not None:
                            # tail(7, jn-1) flushed above -> heads 4-7 done
                            norm_run(jn - 1, dsbB_prev, [4, 5, 6, 7], "B")
                        if h == 5:
                            # tail(3, jn) flushed at h=4 -> heads 0-3 done
                            norm_run(jn, dsbA, [0, 1, 2, 3], "A")
                        if last and h == 7:
                            # tails 4,5 flushed at h=5,6
                            norm_run(jn, dsbC, [4, 5], "C")
                        if h < 4:
                            if exq:
                                exq.pop(0)()
                                if not exq:
                                    # last qproj(jn+1) chunk issued ->
                                    # x(jn+2) DMA can queue behind its reads
                                    xh_next = (load_x(jn + 2)
                                               if jn + 2 < NJ else None)
                        elif exo:
                            exo.pop(0)()
                    dsbB_prev = dsbB
                attn_tail(*pend)
                nsl_l = slice((NJ - 1) * 512, NJ * 512)
                for h in (6, 7):
                    t, r0 = h // 2, (h % 2) * 64
                    nc.vector.tensor_mul(ao[t][r0:r0 + 64, nsl_l],
                                         ao[t][r0:r0 + 64, nsl_l],
                                         pbd[h][r0:r0 + 64, :])
                hold()

            with nc.named_scope("oproj"):
                for c in make_oproj(NJ - 1):
                    c()

    nc.compile()
    return nc


_NC_CACHE = None


def kernel(x, context, Wq, Wk, Wv, Wo, bo, _trace=False):
    global _NC_CACHE, LAST_RESULTS
    x = np.asarray(x, np.float32)
    context = np.asarray(context, np.float32)
    scale = np.float32(DH ** -0.5)

    if _NC_CACHE is None:
        _NC_CACHE = build_nc()
    nc = _NC_CACHE

    import ml_dtypes
    bf16 = ml_dtypes.bfloat16

    def _b(a):
        return np.ascontiguousarray(np.asarray(a, np.float32)).astype(bf16)

    in_maps = []
    for c in range(NC):
        b, g = c // 2, c % 2
        sl = slice(g * IS, (g + 1) * IS)
        m = {
            "xT": _b(x[b].T),
            "cT": _b(context[b].T),
            "wq": _b(np.asarray(Wq, np.float32)[:, sl] * scale),
            "wk": _b(np.asarray(Wk, np.float32)[:, sl]),
            "wv": _b(np.asarray(Wv, np.float32)[:, sl]),
            "wo": _b(np.asarray(Wo, np.float32)[sl, :]),
        }
        in_maps.append(m)
    res = run_bass_kernel_spmd(nc, in_maps, core_ids=list(range(NC)),
                               trace=_trace)
    LAST_RESULTS = res
    out = np.empty((B, N, QD), np.float32)
    bo32 = np.asarray(bo, np.float32)
    for b in range(B):
        out[b] = res.results[2 * b]["out"] + res.results[2 * b + 1]["out"] + bo32
    return out
